# revision 37
# baseline (speedup 1.0000x reference)
"""Trainium2 Bass kernel for DepthConditionModel (depth -> normalized BEV histogram).

Math (per image): bin i = floor(128 + d*A_u), A_u = (u-320)/400; bin j =
floor(128 + d*B_v), B_v = (v-240)/340; BEV[i,j] = pixel count; output is
(BEV - mean)/std (ddof=1).  Camera geometry bounds every point to
i in [48,207], j in [57,198], so the mask/clip in the reference never bind.

Implementation: windowed survival-count matmul histogram, data-parallel
(2 images/core x 8 cores, no collectives).

  * Depth loads as natural [128 rows, 640 cols] tiles (contiguous DMA).
  * t16 = RNE(d*A + 255.5) as int16 on ACT (func=Copy + bias): an exact
    floor(d*A) + 256 except at exact odd integers (measure-zero; the common
    exact value t=0 lands on even 256).  Do NOT use func=Identity with an
    int16 output: it faults the exec unit (NRT_EXEC_UNIT_UNRECOVERABLE).
  * S_j step matrices: one tensor_scalar(is_ge, immediate) per j-threshold
    over the full 640-column width -> DVE 4x mode (0.26 ns/free-elem).
    The [128, wjt, 640] S_j tile is single-buffered (96 KB worst block).
  * S_i step matrices: tensor_tensor against replicated int16 threshold
    tables in u-batches of 32 (DVE 2x mode), with a few batches per block
    offloaded to GpSimd (subtract+clamp, exact in bf16 for these small
    ints) to use spare Pool cycles.
  * One TensorE matmul per image column accumulates T = S_j^T S_i in PSUM.
    Left columns use is_lt, right is_ge: a window may only be clipped on
    its all-zeros side, so each half anchors at the centre bins.
  * The issue stream is software-pipelined one row-block ahead (depth DMA,
    ACT t16 conversions and GpSimd prebuilds for block V+1 are emitted
    before block V's matmul batches) so the in-order DVE/ACT streams never
    serialize block transitions.
  * 2D finite differences of T (GpSimd + DVE), PE transpose back, exact
    integer counts, mean=4.6875 (exact), var via sum(x^2) matmul-ones,
    Sqrt on ACT (table preloaded at startup) + DVE reciprocal, scale/bias
    broadcast via a K=1 PE matmul (no DRAM bounce), border fill, output
    DMAs.

Known sim/HW divergence: CoreSim truncates fp32->int conversions, hardware
rounds to nearest even -- test.py (hardware path) is authoritative:
rel err 9.4e-4 vs the jax reference.
"""

import os
import numpy as np
import ml_dtypes

import concourse.bass as bass
import concourse.bacc as bacc
import concourse.mybir as mybir
import concourse.tile as tile
from concourse.bass_utils import run_bass_kernel_spmd

F32 = mybir.dt.float32
BF16 = mybir.dt.bfloat16
I16 = mybir.dt.int16

# ---------------------------------------------------------------- geometry
H = int(os.environ.get("DK_H", 480))
W = int(os.environ.get("DK_W", 640))
B_TOTAL = 16
N_CORES = 8
B_PER_CORE = int(os.environ.get("DK_BPC", B_TOTAL // N_CORES))
FX, FY = 1000.0, 850.0
CX = float(os.environ.get("DK_CX", 320.0))
CY = float(os.environ.get("DK_CY", 240.0))
GRID = 256
NVOX = GRID * GRID
MU = float(H * W) / NVOX  # exact in fp32 for the real shape (4.6875)

# i-axis (from u): bin = floor(128 + d*A_u)
A_HOST = (np.arange(W, dtype=np.float64) - CX) / (FX * 0.4)  # (u-320)/400
# j-axis (from v): bin = floor(128 + d*B_v)
B_HOST = (np.arange(H, dtype=np.float64) - CY) / (FY * 0.4)  # (v-240)/340

DMAX = 100.0
A32 = ((np.arange(W, dtype=np.float32) - np.float32(CX)) / np.float32(FX * 0.4))
B32 = ((np.arange(H, dtype=np.float32) - np.float32(CY)) / np.float32(FY * 0.4))

# per-u i-bin windows (with +-1 safety margin)
I_LO = np.floor(128.0 + DMAX * np.minimum(0.0, A_HOST)).astype(np.int64) - 1
I_HI = np.floor(128.0 + DMAX * np.maximum(0.0, A_HOST)).astype(np.int64) + 1

LO_BIN0 = int(I_LO.min())       # 47
LO_BIN1 = 130                   # left windows end at bin 129 (+1 margin)
HI_BIN0 = 127                   # right windows start at bin 128 (-1 margin)
HI_BIN1 = int(I_HI.max()) + 1   # 209

N_BLK = (H + 127) // 128
BLK_V0 = [128 * V for V in range(N_BLK)]
BLK_ROWS = [min(128, H - v0) for v0 in BLK_V0]
J_LO, J_HI = [], []
for V in range(N_BLK):
    bs = B_HOST[BLK_V0[V] : BLK_V0[V] + BLK_ROWS[V]]
    J_LO.append(int(np.floor(128.0 + DMAX * min(0.0, bs.min()))) - 1)
    J_HI.append(int(np.floor(128.0 + DMAX * max(0.0, bs.max()))) + 1)
JLO_G = min(J_LO)   # 56
JHI_G = max(J_HI)   # 199
NJ = JHI_G - JLO_G + 1  # 144 output columns [56..199]

BATCH = int(os.environ.get("DK_BATCH", 32))  # u-columns per DVE instruction
WJT_MAX = max(J_HI[V] - J_LO[V] + 2 for V in range(N_BLK))
U_SPLIT = int(np.searchsorted(A_HOST, 0.0))  # columns < U_SPLIT are "left"
U_SPLIT = ((U_SPLIT + BATCH - 1) // BATCH) * BATCH  # align to batch boundary
assert 0 < U_SPLIT < W and U_SPLIT % BATCH == 0 and W % BATCH == 0
assert np.all(A_HOST[:U_SPLIT] * DMAX < 1.0), "left-side columns must stay below bin 130"

# threshold tables; threshold value = bin + 128, compared against
# t16 = rne(d*A + 255.5) (int16; RNE(x-0.5) is an exact floor except at
# exact odd integers, which are measure-zero here)
THRL_VALS = (np.arange(LO_BIN0, LO_BIN1 + 1) + 128).astype(np.int16)
THRH_VALS = (np.arange(HI_BIN0, HI_BIN1 + 1) + 128).astype(np.int16)
THRJ_VALS = (np.arange(JLO_G, JHI_G + 2) + 128).astype(np.int16)
N_THRL = len(THRL_VALS)   # 84
N_THRH = len(THRH_VALS)   # 83
N_THRJ = len(THRJ_VALS)   # 145

NBL = LO_BIN1 - LO_BIN0   # 83: BEVL bins [47..129]
NBR = HI_BIN1 - HI_BIN0   # 82: BEVR bins [127..208]


def _make_consts():
    """Constant input arrays (replicated across partitions where needed)."""
    consts = {}
    consts["a_tile"] = np.broadcast_to(A32[None, :], (128, W)).copy()
    bcol = np.zeros((128, N_BLK), np.float32)
    for V in range(N_BLK):
        bcol[: BLK_ROWS[V], V] = B32[BLK_V0[V] : BLK_V0[V] + BLK_ROWS[V]]
    consts["b_col"] = bcol
    # threshold tables pre-replicated BATCH times along the free dim
    consts["thr_l_rep"] = np.broadcast_to(
        np.repeat(THRL_VALS, BATCH)[None, :], (128, N_THRL * BATCH)
    ).copy()
    consts["thr_h_rep"] = np.broadcast_to(
        np.repeat(THRH_VALS, BATCH)[None, :], (128, N_THRH * BATCH)
    ).copy()
    consts["thr_h1_rep"] = (consts["thr_h_rep"] - 1).astype(np.int16)
    consts["ones_c"] = np.ones((128, 1), np.float32)
    consts["ones_row"] = np.ones((1, 128), np.float32)
    consts["zeros_b"] = np.zeros((128, 128), ml_dtypes.bfloat16)
    consts["ident"] = np.eye(128, dtype=np.float32)
    return consts


def build_program(n_img=B_PER_CORE):
    nc = bacc.Bacc("TRN2", target_bir_lowering=False, debug=False)

    depth_in = nc.dram_tensor("depth", [n_img, H, W], F32, kind="ExternalInput").ap()
    a_in = nc.dram_tensor("a_tile", [128, W], F32, kind="ExternalInput").ap()
    bcol_in = nc.dram_tensor("b_col", [128, N_BLK], F32, kind="ExternalInput").ap()
    thrlr_in = nc.dram_tensor("thr_l_rep", [128, N_THRL * BATCH], I16, kind="ExternalInput").ap()
    thrhr_in = nc.dram_tensor("thr_h_rep", [128, N_THRH * BATCH], I16, kind="ExternalInput").ap()
    thrh1r_in = nc.dram_tensor("thr_h1_rep", [128, N_THRH * BATCH], I16, kind="ExternalInput").ap()
    ones_in = nc.dram_tensor("ones_c", [128, 1], F32, kind="ExternalInput").ap()
    onesr_in = nc.dram_tensor("ones_row", [1, 128], F32, kind="ExternalInput").ap()
    zeros_in = nc.dram_tensor("zeros_b", [128, 128], BF16, kind="ExternalInput").ap()
    ident_in = nc.dram_tensor("ident", [128, 128], F32, kind="ExternalInput").ap()
    out_dram = nc.dram_tensor("bev_out", [n_img, GRID, GRID], F32, kind="ExternalOutput").ap()

    n_repeat = int(os.environ.get("DK_REPEAT", 1))
    imgs = [i for _ in range(n_repeat) for i in range(n_img)]
    stages = [(img, V) for img in imgs for V in range(N_BLK)]
    # batch positions (within each side's issue order) offloaded to GpSimd
    pool_idx = {
        int(s) for s in os.environ.get("DK_POOL_IDX", "2,7").split(",") if s
    }

    with tile.TileContext(nc) as tc:
        with (
            tc.tile_pool(name="const", bufs=1) as cp,
            tc.tile_pool(name="work", bufs=2) as wp,
            tc.tile_pool(name="sbatch", bufs=4) as sbp,
            tc.tile_pool(name="poolsi", bufs=4) as psip,
            tc.tile_pool(name="sjpool", bufs=1) as sjp,
            tc.tile_pool(name="acc", bufs=2) as accp,
            tc.tile_pool(name="post", bufs=2) as postp,
            tc.tile_pool(name="psL", bufs=2, space="PSUM") as ppL,
            tc.tile_pool(name="psR", bufs=2, space="PSUM") as ppR,
            tc.tile_pool(name="psT", bufs=2, space="PSUM") as ppT,
            tc.tile_pool(name="psS", bufs=1, space="PSUM") as ppS,
        ):
            # ---------------- constants + stage-0 prologue head ----------
            a_t = cp.tile([128, W], F32, tag="a")
            bcol_t = cp.tile([128, N_BLK], F32, tag="b")
            ones_t = cp.tile([128, 1], F32, tag="on")
            onesr_t = cp.tile([1, 128], F32, tag="onr")
            zeros_t = cp.tile([128, 128], BF16, tag="zb")
            ident_t = cp.tile([128, 128], F32, tag="id")
            dummy_t = cp.tile([1, 1], F32, tag="dmy")
            thrl_r = cp.tile([128, N_THRL * BATCH], I16, tag="tlr")
            thrh_r = cp.tile([128, N_THRH * BATCH], I16, tag="thr")
            thrh1_r = cp.tile([128, N_THRH * BATCH], I16, tag="thr1")

            # stage-0 depth DMA first so it overlaps the const loads
            pro = {}  # stage -> dict(d, ti, tj, pool_views)
            d0 = wp.tile([128, W], F32, tag="d")
            img0, V0 = stages[0]
            nc.sync.dma_start(
                out=d0[: BLK_ROWS[V0], :],
                in_=depth_in[img0, BLK_V0[V0] : BLK_V0[V0] + BLK_ROWS[V0], :],
            )

            nc.sync.dma_start(out=bcol_t[:], in_=bcol_in[:])
            nc.sync.dma_start(out=a_t[:], in_=a_in[:])
            nc.sync.dma_start(out=thrl_r[:], in_=thrlr_in[:])
            nc.sync.dma_start(out=thrh_r[:], in_=thrhr_in[:])
            nc.sync.dma_start(out=thrh1_r[:], in_=thrh1r_in[:])
            nc.sync.dma_start(out=ones_t[:], in_=ones_in[:])
            nc.sync.dma_start(out=onesr_t[:], in_=onesr_in[:])
            nc.sync.dma_start(out=zeros_t[:], in_=zeros_in[:])
            nc.sync.dma_start(out=ident_t[:], in_=ident_in[:])
            # preload the Sqrt activation table off the critical path
            nc.vector.memset(dummy_t[:], 1.0)
            nc.scalar.activation(dummy_t[:], dummy_t[:], mybir.ActivationFunctionType.Sqrt)

            def pool_si_build(t_b, thr_v, K, wb, ge):
                """GpSimd step-matrix build: subtract + clamp in 16-u quanta
                (finer quanta shorten PE stalls).  Returns the si view."""
                tmp_b = psip.tile([128, BATCH * (N_THRL + 2)], BF16, tag="gptmp")
                tmp_v = tmp_b[:K, : BATCH * wb].rearrange("p (w c) -> p w c", c=BATCH)
                for c0 in range(0, BATCH, 16):
                    if ge:  # 1[t >= thr] = clamp(t - (thr-1), 0, 1)
                        nc.gpsimd.tensor_tensor(
                            out=tmp_v[:, :, c0 : c0 + 16],
                            in0=t_b[:, :, c0 : c0 + 16],
                            in1=thr_v[:, :, c0 : c0 + 16],
                            op=mybir.AluOpType.subtract,
                        )
                    else:  # 1[t < thr] = clamp(thr - t, 0, 1)
                        nc.gpsimd.tensor_tensor(
                            out=tmp_v[:, :, c0 : c0 + 16],
                            in0=thr_v[:, :, c0 : c0 + 16],
                            in1=t_b[:, :, c0 : c0 + 16],
                            op=mybir.AluOpType.subtract,
                        )
                    nc.gpsimd.tensor_scalar(
                        out=tmp_v[:, :, c0 : c0 + 16], in0=tmp_v[:, :, c0 : c0 + 16],
                        scalar1=0.0, scalar2=1.0,
                        op0=mybir.AluOpType.max, op1=mybir.AluOpType.min,
                    )
                return tmp_v

            def left_batches(V):
                out = []
                for bi, b0 in enumerate(range(0, U_SPLIT, BATCH)):
                    lo_min = int(I_LO[b0 : b0 + BATCH].min())
                    out.append((bi, b0, LO_BIN1 - lo_min + 1, lo_min))
                return out

            def right_batches(V):
                out = []
                for bi, b0 in enumerate(range(W - BATCH, U_SPLIT - 1, -BATCH)):
                    hi_max = int(I_HI[b0 : b0 + BATCH].max())
                    out.append((bi, b0, hi_max + 1 - HI_BIN0 + 1, hi_max))
                return out

            def emit_prologue_head(s):
                """Depth DMA + ACT tj for stage s (cheap, issue early)."""
                img, V = stages[s]
                K = BLK_ROWS[V]
                if s == 0:
                    d_t = d0
                else:
                    d_t = wp.tile([128, W], F32, tag="d")
                    nc.sync.dma_start(
                        out=d_t[:K, :], in_=depth_in[img, BLK_V0[V] : BLK_V0[V] + K, :]
                    )
                tj_t = wp.tile([128, W], I16, tag="tj")
                # ACT Copy: out = in*scale + 255.5, int16 RNE convert on write
                nc.scalar.activation(
                    tj_t[:K, :], d_t[:K, :],
                    mybir.ActivationFunctionType.Copy, bias=255.5,
                    scale=bcol_t[:K, V : V + 1],
                )
                pro[s] = {"d": d_t, "tj": tj_t}

            def emit_prologue_tail(s):
                """tif mult (DVE), ti ACT, GpSimd si prebuilds for stage s."""
                img, V = stages[s]
                K = BLK_ROWS[V]
                d_t = pro[s]["d"]
                tif_t = wp.tile([128, W], F32, tag="tif")
                nc.vector.tensor_tensor(
                    out=tif_t[:K, :], in0=d_t[:K, :], in1=a_t[:K, :],
                    op=mybir.AluOpType.mult,
                )
                ti_t = wp.tile([128, W], I16, tag="ti")
                nc.scalar.activation(
                    ti_t[:K, :], tif_t[:K, :],
                    mybir.ActivationFunctionType.Copy, bias=255.5,
                )
                pro[s]["ti"] = ti_t
                pool_views = {}
                stage_pool_idx = set() if V == 0 else pool_idx
                for bi, b0, wb, lo_min in left_batches(V):
                    if bi not in stage_pool_idx:
                        continue
                    t_b = ti_t[:K, b0 : b0 + BATCH].unsqueeze(1).to_broadcast([K, wb, BATCH])
                    thr_v = thrl_r[
                        :K, (lo_min - LO_BIN0) * BATCH : (lo_min - LO_BIN0 + wb) * BATCH
                    ].rearrange("p (w c) -> p w c", c=BATCH)
                    pool_views[("L", bi)] = pool_si_build(t_b, thr_v, K, wb, ge=False)
                for bi, b0, wb, hi_max in right_batches(V):
                    if bi not in stage_pool_idx:
                        continue
                    t_b = ti_t[:K, b0 : b0 + BATCH].unsqueeze(1).to_broadcast([K, wb, BATCH])
                    thr1_v = thrh1_r[:K, : wb * BATCH].rearrange("p (w c) -> p w c", c=BATCH)
                    pool_views[("R", bi)] = pool_si_build(t_b, thr1_v, K, wb, ge=True)
                pro[s]["pool_views"] = pool_views

            emit_prologue_head(0)
            emit_prologue_tail(0)

            NROW_L = HI_BIN0 - LO_BIN0  # 80 rows: bins [47..126]
            cur_bev = {}

            def emit_diffs(V, lt, ht, wjt, joff, on_dve=False):
                """2D finite differences for one block: i-diff along free,
                PE transpose, then j-diff along free -- no cross-partition
                shift DMA.  The last 3 left i-bins [127..129] overlap BEVR
                and are added there.  Mid-image blocks run on GpSimd; the
                image's last block runs on DVE so the stats chain does not
                queue behind GpSimd prebuilds."""
                bevl, bevr = cur_bev["l"], cur_bev["r"]
                nbj = wjt - 1  # j-bins in this block
                eng = nc.vector if on_dve else nc.gpsimd

                u_d = postp.tile([WJT_MAX, N_THRL], F32, tag="ud")
                eng.tensor_tensor(
                    out=u_d[:wjt, : N_THRL - 1],
                    in0=lt[:wjt, 1:N_THRL], in1=lt[:wjt, : N_THRL - 1],
                    op=mybir.AluOpType.subtract,
                )
                pst = ppT.tile([NBL, 2 * WJT_MAX], F32, tag="pt")
                nc.tensor.transpose(
                    out=pst[: N_THRL - 1, :wjt],
                    in_=u_d[:wjt, : N_THRL - 1],
                    identity=ident_t[:wjt, :wjt],
                )
                udT = postp.tile([NBL, WJT_MAX], F32, tag="udT")
                nc.scalar.copy(out=udT[:NROW_L, :wjt], in_=pst[:NROW_L, :wjt])
                dj = postp.tile([NBL, WJT_MAX], F32, tag="dj")
                eng.tensor_tensor(
                    out=dj[:NROW_L, :nbj],
                    in0=udT[:NROW_L, :nbj], in1=udT[:NROW_L, 1:wjt],
                    op=mybir.AluOpType.subtract,
                )
                eng.tensor_tensor(
                    out=bevl[:, joff : joff + nbj],
                    in0=bevl[:, joff : joff + nbj],
                    in1=dj[:NROW_L, :nbj],
                    op=mybir.AluOpType.add,
                )
                # overlap bins [127..129]: transpose those 3 columns to a
                # partition-0-based tile (compute slices must start 32-aligned)
                nc.tensor.transpose(
                    out=pst[0:3, WJT_MAX : WJT_MAX + wjt],
                    in_=u_d[:wjt, NROW_L : N_THRL - 1],
                    identity=ident_t[:wjt, :wjt],
                )
                udT2 = postp.tile([32, WJT_MAX], F32, tag="udT2")
                nc.scalar.copy(out=udT2[0:3, :wjt], in_=pst[0:3, WJT_MAX : WJT_MAX + wjt])
                dj2 = postp.tile([32, WJT_MAX], F32, tag="dj2")
                eng.tensor_tensor(
                    out=dj2[0:3, :nbj],
                    in0=udT2[0:3, :nbj], in1=udT2[0:3, 1:wjt],
                    op=mybir.AluOpType.subtract,
                )
                eng.tensor_tensor(
                    out=bevr[0:3, joff : joff + nbj],
                    in0=bevr[0:3, joff : joff + nbj],
                    in1=dj2[0:3, :nbj],
                    op=mybir.AluOpType.add,
                )

                u_r = postp.tile([WJT_MAX, N_THRH], F32, tag="ur")
                eng.tensor_tensor(
                    out=u_r[:wjt, : N_THRH - 1],
                    in0=ht[:wjt, : N_THRH - 1], in1=ht[:wjt, 1:N_THRH],
                    op=mybir.AluOpType.subtract,
                )
                pst_r = ppT.tile([NBR, 2 * WJT_MAX], F32, tag="pt")
                nc.tensor.transpose(
                    out=pst_r[: N_THRH - 1, :wjt],
                    in_=u_r[:wjt, : N_THRH - 1],
                    identity=ident_t[:wjt, :wjt],
                )
                urT = postp.tile([NBR, WJT_MAX], F32, tag="urT")
                nc.scalar.copy(out=urT[: N_THRH - 1, :wjt], in_=pst_r[: N_THRH - 1, :wjt])
                dj_r = postp.tile([NBR, WJT_MAX], F32, tag="djr")
                eng.tensor_tensor(
                    out=dj_r[: N_THRH - 1, :nbj],
                    in0=urT[: N_THRH - 1, :nbj], in1=urT[: N_THRH - 1, 1:wjt],
                    op=mybir.AluOpType.subtract,
                )
                eng.tensor_tensor(
                    out=bevr[:, joff : joff + nbj],
                    in0=bevr[:, joff : joff + nbj],
                    in1=dj_r[: N_THRH - 1, :nbj],
                    op=mybir.AluOpType.add,
                )

            def emit_post(img):
                """Statistics, normalize, output DMAs."""
                bevl, bevr = cur_bev["l"], cur_bev["r"]
                nrow_l = NROW_L             # 80 rows: bins [47..126]
                nrow_r = NBR                # 82 rows: bins [127..208]
                sq = postp.tile([128, NJ], F32, tag="sq")
                sql = postp.tile([128, 1], F32, tag="sql")
                sqr = postp.tile([128, 1], F32, tag="sqr")
                nc.vector.tensor_tensor(
                    out=sq[:nrow_l, :], in0=bevl[:nrow_l, :], in1=bevl[:nrow_l, :],
                    op=mybir.AluOpType.mult,
                )
                nc.vector.tensor_reduce(
                    out=sql[:nrow_l, :], in_=sq[:nrow_l, :],
                    axis=mybir.AxisListType.X, op=mybir.AluOpType.add,
                )
                nc.vector.tensor_tensor(
                    out=sq[:nrow_r, :], in0=bevr[:nrow_r, :], in1=bevr[:nrow_r, :],
                    op=mybir.AluOpType.mult,
                )
                nc.vector.tensor_reduce(
                    out=sqr[:nrow_r, :], in_=sq[:nrow_r, :],
                    axis=mybir.AxisListType.X, op=mybir.AluOpType.add,
                )
                pss = ppS.tile([1, 1], F32, tag="ps")
                nc.tensor.matmul(
                    pss[:, :], lhsT=sql[:nrow_l, :], rhs=ones_t[:nrow_l, :],
                    start=True, stop=False,
                )
                nc.tensor.matmul(
                    pss[:, :], lhsT=sqr[:nrow_r, :], rhs=ones_t[:nrow_r, :],
                    start=False, stop=True,
                )
                ib_pair = postp.tile([1, 2], F32, tag="ibp")
                var_t = postp.tile([1, 1], F32, tag="var")
                nc.vector.tensor_scalar(
                    out=var_t[:], in0=pss[:, :],
                    scalar1=-float(NVOX) * MU * MU, scalar2=1.0 / float(NVOX - 1),
                    op0=mybir.AluOpType.add, op1=mybir.AluOpType.mult,
                )
                std_t = postp.tile([1, 1], F32, tag="std")
                nc.scalar.activation(std_t[:], var_t[:], mybir.ActivationFunctionType.Sqrt)
                nc.vector.reciprocal(ib_pair[:, 0:1], std_t[:])
                nc.vector.tensor_scalar(
                    out=ib_pair[:, 1:2], in0=ib_pair[:, 0:1], scalar1=-MU, scalar2=None,
                    op0=mybir.AluOpType.mult,
                )
                # broadcast (inv, bias) across partitions via a K=1 matmul
                bc_ps = ppS.tile([128, 2], F32, tag="bc")
                nc.tensor.matmul(
                    bc_ps[:, :], lhsT=onesr_t[0:1, :], rhs=ib_pair[0:1, :],
                    start=True, stop=True,
                )
                invb = postp.tile([128, 2], F32, tag="invb")
                nc.scalar.copy(out=invb[:], in_=bc_ps[:, :])

                # ---- normalize + write out ----
                bevln = postp.tile([NBL, NJ], F32, tag="bevln")
                bevrn = postp.tile([NBR, NJ], F32, tag="bevrn")
                nc.vector.tensor_scalar(
                    out=bevln[:nrow_l, :], in0=bevl[:nrow_l, :],
                    scalar1=invb[:nrow_l, 0:1], scalar2=invb[:nrow_l, 1:2],
                    op0=mybir.AluOpType.mult, op1=mybir.AluOpType.add,
                )
                nc.vector.tensor_scalar(
                    out=bevrn[:nrow_r, :], in0=bevr[:nrow_r, :],
                    scalar1=invb[:nrow_r, 0:1], scalar2=invb[:nrow_r, 1:2],
                    op0=mybir.AluOpType.mult, op1=mybir.AluOpType.add,
                )
                border = postp.tile([128, GRID], F32, tag="border")
                nc.gpsimd.memset(border[:], 0.0)
                nc.vector.tensor_scalar(
                    out=border[:], in0=border[:], scalar1=invb[:, 1:2], scalar2=None,
                    op0=mybir.AluOpType.add,
                )

                r0 = LO_BIN0  # first computed row (47)
                r1 = HI_BIN0  # 127
                r2 = HI_BIN0 + NBR  # 209
                nc.sync.dma_start(
                    out=out_dram[img, r0:r1, JLO_G : JLO_G + NJ], in_=bevln[: r1 - r0, :]
                )
                nc.sync.dma_start(
                    out=out_dram[img, r1:r2, JLO_G : JLO_G + NJ], in_=bevrn[: r2 - r1, :]
                )

                def border_fill(rr0, rr1, cc0, cc1):
                    r = rr0
                    while r < rr1:
                        n = min(128, rr1 - r)
                        nc.sync.dma_start(
                            out=out_dram[img, r : r + n, cc0:cc1],
                            in_=border[:n, : cc1 - cc0],
                        )
                        r += n

                border_fill(0, r0, 0, GRID)
                border_fill(r2, GRID, 0, GRID)
                border_fill(r0, r2, 0, JLO_G)
                border_fill(r0, r2, JLO_G + NJ, GRID)

            # ------------------------------ main pipelined stage loop ----
            for s, (img, V) in enumerate(stages):
                if V == 0:
                    bevl = postp.tile([NROW_L, NJ], F32, tag="bevl")
                    bevr = postp.tile([NBR, NJ], F32, tag="bevr")
                    nc.gpsimd.memset(bevl[:], 0.0)
                    nc.gpsimd.memset(bevr[:], 0.0)
                    cur_bev["l"], cur_bev["r"] = bevl, bevr
                K = BLK_ROWS[V]
                wjt = J_HI[V] - J_LO[V] + 2  # j-threshold count
                joff = J_LO[V] - JLO_G       # global j offset of this block
                ti_t = pro[s]["ti"]
                tj_t = pro[s]["tj"]
                pool_views = pro[s]["pool_views"]

                # next stage's DMA + tj go out before this block's work
                if s + 1 < len(stages):
                    emit_prologue_head(s + 1)

                psL = ppL.tile([WJT_MAX, N_THRL], F32, tag="pl")
                psH = ppR.tile([WJT_MAX, N_THRH], F32, tag="pr")

                # ---- S_j build (DVE 4x), full block width per threshold ----
                sj_t = sjp.tile([128, WJT_MAX * W], BF16, tag="sj")
                sj_f = sj_t[:].rearrange("p (t u) -> p t u", u=W)
                for jt in range(wjt):
                    nc.vector.tensor_scalar(
                        out=sj_f[:K, jt, :],
                        in0=tj_t[:K, :],
                        scalar1=int(THRJ_VALS[joff + jt]),
                        scalar2=None,
                        op0=mybir.AluOpType.is_ge,
                    )

                # rest of next stage's prologue (tif/ti/pool prebuilds)
                if s + 1 < len(stages):
                    emit_prologue_tail(s + 1)

                # ---- left columns (u < U_SPLIT): S_i = (t < thr) ----
                for bi, b0, wb, lo_min in left_batches(V):
                    us = range(b0, b0 + BATCH)
                    if ("L", bi) in pool_views:
                        si_v = pool_views[("L", bi)]
                    else:
                        t_b = ti_t[:K, b0 : b0 + BATCH].unsqueeze(1).to_broadcast([K, wb, BATCH])
                        thr_v = thrl_r[
                            :K, (lo_min - LO_BIN0) * BATCH : (lo_min - LO_BIN0 + wb) * BATCH
                        ].rearrange("p (w c) -> p w c", c=BATCH)
                        si_b = sbp.tile([128, BATCH * (N_THRL + 2)], BF16, tag="si")
                        si_v = si_b[:K, : BATCH * wb].rearrange("p (w c) -> p w c", c=BATCH)
                        nc.vector.tensor_tensor(
                            out=si_v, in0=t_b, in1=thr_v, op=mybir.AluOpType.is_lt,
                        )
                    for c, u in enumerate(us):
                        wi = LO_BIN1 - int(I_LO[u]) + 1
                        foff = int(I_LO[u]) - LO_BIN0
                        nc.tensor.matmul(
                            psL[:wjt, foff : foff + wi],
                            lhsT=sj_f[:K, :wjt, u],
                            rhs=si_v[:, wb - wi :, c],
                            start=(u == 0),
                            stop=False,
                        )

                # close the left accumulation group with a full-size
                # zero matmul (stop is sim-only bookkeeping)
                nc.tensor.matmul(
                    psL[:wjt, :],
                    lhsT=zeros_t[:K, :wjt],
                    rhs=zeros_t[:K, :N_THRL],
                    start=False, stop=True,
                )

                # ---- right columns (u >= U_SPLIT), descending so the
                # widest window (u = W-1) opens the group ----
                for bi, b0, wb, hi_max in right_batches(V):
                    us = range(b0, b0 + BATCH)
                    if ("R", bi) in pool_views:
                        si_v = pool_views[("R", bi)]
                    else:
                        t_b = ti_t[:K, b0 : b0 + BATCH].unsqueeze(1).to_broadcast([K, wb, BATCH])
                        si_b = sbp.tile([128, BATCH * (N_THRL + 2)], BF16, tag="si")
                        si_v = si_b[:K, : BATCH * wb].rearrange("p (w c) -> p w c", c=BATCH)
                        nc.vector.tensor_tensor(
                            out=si_v,
                            in0=t_b,
                            in1=thrh_r[:K, : wb * BATCH].rearrange("p (w c) -> p w c", c=BATCH),
                            op=mybir.AluOpType.is_ge,
                        )
                    for cc, u in enumerate(reversed(us)):
                        c = BATCH - 1 - cc
                        wi = int(I_HI[u]) + 2 - HI_BIN0
                        nc.tensor.matmul(
                            psH[:wjt, :wi],
                            lhsT=sj_f[:K, :wjt, u],
                            rhs=si_v[:, :wi, c],
                            start=(u == W - 1),
                            stop=False,
                        )
                nc.tensor.matmul(
                    psH[:wjt, :],
                    lhsT=zeros_t[:K, :wjt],
                    rhs=zeros_t[:K, :N_THRH],
                    start=False, stop=True,
                )

                lt = accp.tile([WJT_MAX, N_THRL], F32, tag="lt")
                ht = accp.tile([WJT_MAX, N_THRH], F32, tag="ht")
                nc.scalar.copy(out=lt[:wjt, :], in_=psL[:wjt, :])
                nc.scalar.copy(out=ht[:wjt, :], in_=psH[:wjt, :])
                emit_diffs(V, lt, ht, wjt, joff, on_dve=(V == N_BLK - 1))

                if V == N_BLK - 1:
                    emit_post(img)

    nc.compile()
    return nc


_NC_CACHE = {}
LAST_RESULTS = None


def kernel(depth: np.ndarray) -> np.ndarray:
    """Full-input entry point: depth (16, 480, 640) f32 -> (16, 1, 256, 256) f32."""
    global LAST_RESULTS
    depth = np.asarray(depth, dtype=np.float32)
    assert depth.shape == (B_TOTAL, H, W)

    import sys, time as _time
    if "nc" not in _NC_CACHE:
        _t0 = _time.time()
        print("[kernel] building program...", file=sys.stderr, flush=True)
        _NC_CACHE["nc"] = build_program(B_PER_CORE)
        print(f"[kernel] program built in {_time.time()-_t0:.1f}s", file=sys.stderr, flush=True)
    nc = _NC_CACHE["nc"]

    consts = _make_consts()
    in_maps = []
    for c in range(N_CORES):
        m = dict(consts)
        m["depth"] = np.ascontiguousarray(depth[c * B_PER_CORE : (c + 1) * B_PER_CORE])
        in_maps.append(m)

    print("[kernel] launching spmd run...", file=__import__("sys").stderr, flush=True)
    res = run_bass_kernel_spmd(
        nc, in_maps, list(range(N_CORES)),
        trace=bool(os.environ.get("BASS_TRACE")),
    )
    LAST_RESULTS = res
    out = np.empty((B_TOTAL, 1, GRID, GRID), np.float32)
    for c in range(N_CORES):
        out[c * B_PER_CORE : (c + 1) * B_PER_CORE, 0] = res.results[c]["bev_out"]
    return out


# revision 38
# speedup vs baseline: 1.0374x; 1.0374x over previous
"""Trainium2 Bass kernel for DepthConditionModel (depth -> normalized BEV histogram).

Math (per image): bin i = floor(128 + d*A_u), A_u = (u-320)/400; bin j =
floor(128 + d*B_v), B_v = (v-240)/340; BEV[i,j] = pixel count; output is
(BEV - mean)/std (ddof=1).  Camera geometry bounds every point to
i in [48,207], j in [57,198], so the mask/clip in the reference never bind.

Implementation: windowed survival-count matmul histogram, data-parallel
(2 images/core x 8 cores, no collectives).

  * Depth loads as natural [128 rows, 640 cols] tiles (contiguous DMA).
  * t16 = RNE(d*A + 255.5) as int16 on ACT (func=Copy + bias): an exact
    floor(d*A) + 256 except at exact odd integers (measure-zero; the common
    exact value t=0 lands on even 256).  Do NOT use func=Identity with an
    int16 output: it faults the exec unit (NRT_EXEC_UNIT_UNRECOVERABLE).
  * S_j step matrices: one tensor_scalar(is_ge, immediate) per j-threshold
    over the full 640-column width -> DVE 4x mode (0.26 ns/free-elem).
    The [128, wjt, 640] S_j tile is single-buffered (96 KB worst block).
  * S_i step matrices: tensor_tensor against replicated int16 threshold
    tables in u-batches of 32 (DVE 2x mode), with a few batches per block
    offloaded to GpSimd (subtract+clamp, exact in bf16 for these small
    ints) to use spare Pool cycles.
  * One TensorE matmul per image column accumulates T = S_j^T S_i in PSUM.
    Left columns use is_lt, right is_ge: a window may only be clipped on
    its all-zeros side, so each half anchors at the centre bins.
  * The issue stream is software-pipelined one row-block ahead (depth DMA,
    ACT t16 conversions and GpSimd prebuilds for block V+1 are emitted
    before block V's matmul batches) so the in-order DVE/ACT streams never
    serialize block transitions.
  * 2D finite differences of T (GpSimd + DVE), PE transpose back, exact
    integer counts, mean=4.6875 (exact), var via sum(x^2) matmul-ones,
    Sqrt on ACT (table preloaded at startup) + DVE reciprocal, scale/bias
    broadcast via a K=1 PE matmul (no DRAM bounce), border fill, output
    DMAs.

Known sim/HW divergence: CoreSim truncates fp32->int conversions, hardware
rounds to nearest even -- test.py (hardware path) is authoritative:
rel err 9.4e-4 vs the jax reference.
"""

import os
import numpy as np
import ml_dtypes

import concourse.bass as bass
import concourse.bacc as bacc
import concourse.mybir as mybir
import concourse.tile as tile
from concourse.bass_utils import run_bass_kernel_spmd

F32 = mybir.dt.float32
BF16 = mybir.dt.bfloat16
I16 = mybir.dt.int16

# ---------------------------------------------------------------- geometry
H = int(os.environ.get("DK_H", 480))
W = int(os.environ.get("DK_W", 640))
B_TOTAL = 16
N_CORES = 8
B_PER_CORE = int(os.environ.get("DK_BPC", B_TOTAL // N_CORES))
FX, FY = 1000.0, 850.0
CX = float(os.environ.get("DK_CX", 320.0))
CY = float(os.environ.get("DK_CY", 240.0))
GRID = 256
NVOX = GRID * GRID
MU = float(H * W) / NVOX  # exact in fp32 for the real shape (4.6875)

# i-axis (from u): bin = floor(128 + d*A_u)
A_HOST = (np.arange(W, dtype=np.float64) - CX) / (FX * 0.4)  # (u-320)/400
# j-axis (from v): bin = floor(128 + d*B_v)
B_HOST = (np.arange(H, dtype=np.float64) - CY) / (FY * 0.4)  # (v-240)/340

DMAX = 100.0
A32 = ((np.arange(W, dtype=np.float32) - np.float32(CX)) / np.float32(FX * 0.4))
B32 = ((np.arange(H, dtype=np.float32) - np.float32(CY)) / np.float32(FY * 0.4))

# per-u i-bin windows (with +-1 safety margin)
I_LO = np.floor(128.0 + DMAX * np.minimum(0.0, A_HOST)).astype(np.int64) - 1
I_HI = np.floor(128.0 + DMAX * np.maximum(0.0, A_HOST)).astype(np.int64) + 1

LO_BIN0 = int(I_LO.min())       # 47
LO_BIN1 = 130                   # left windows end at bin 129 (+1 margin)
HI_BIN0 = 127                   # right windows start at bin 128 (-1 margin)
HI_BIN1 = int(I_HI.max()) + 1   # 209

N_BLK = (H + 127) // 128
BLK_V0 = [128 * V for V in range(N_BLK)]
BLK_ROWS = [min(128, H - v0) for v0 in BLK_V0]
J_LO, J_HI = [], []
for V in range(N_BLK):
    bs = B_HOST[BLK_V0[V] : BLK_V0[V] + BLK_ROWS[V]]
    J_LO.append(int(np.floor(128.0 + DMAX * min(0.0, bs.min()))) - 1)
    J_HI.append(int(np.floor(128.0 + DMAX * max(0.0, bs.max()))) + 1)
JLO_G = min(J_LO)   # 56
JHI_G = max(J_HI)   # 199
NJ = JHI_G - JLO_G + 1  # 144 output columns [56..199]

BATCH = int(os.environ.get("DK_BATCH", 32))  # u-columns per DVE instruction
WJT_MAX = max(J_HI[V] - J_LO[V] + 2 for V in range(N_BLK))
U_SPLIT = int(np.searchsorted(A_HOST, 0.0))  # columns < U_SPLIT are "left"
U_SPLIT = ((U_SPLIT + BATCH - 1) // BATCH) * BATCH  # align to batch boundary
assert 0 < U_SPLIT < W and U_SPLIT % BATCH == 0 and W % BATCH == 0
assert np.all(A_HOST[:U_SPLIT] * DMAX < 1.0), "left-side columns must stay below bin 130"

# threshold tables; threshold value = bin + 128, compared against
# t16 = rne(d*A + 255.5) (int16; RNE(x-0.5) is an exact floor except at
# exact odd integers, which are measure-zero here)
THRL_VALS = (np.arange(LO_BIN0, LO_BIN1 + 1) + 128).astype(np.int16)
THRH_VALS = (np.arange(HI_BIN0, HI_BIN1 + 1) + 128).astype(np.int16)
THRJ_VALS = (np.arange(JLO_G, JHI_G + 2) + 128).astype(np.int16)
N_THRL = len(THRL_VALS)   # 84
N_THRH = len(THRH_VALS)   # 83
N_THRJ = len(THRJ_VALS)   # 145

NBL = LO_BIN1 - LO_BIN0   # 83: BEVL bins [47..129]
NBR = HI_BIN1 - HI_BIN0   # 82: BEVR bins [127..208]


def _make_consts():
    """Constant input arrays (replicated across partitions where needed)."""
    consts = {}
    consts["a_tile"] = np.broadcast_to(A32[None, :], (128, W)).copy()
    bcol = np.zeros((128, N_BLK), np.float32)
    for V in range(N_BLK):
        bcol[: BLK_ROWS[V], V] = B32[BLK_V0[V] : BLK_V0[V] + BLK_ROWS[V]]
    consts["b_col"] = bcol
    # threshold tables pre-replicated BATCH times along the free dim
    consts["thr_l_rep"] = np.broadcast_to(
        np.repeat(THRL_VALS, BATCH)[None, :], (128, N_THRL * BATCH)
    ).copy()
    consts["thr_h_rep"] = np.broadcast_to(
        np.repeat(THRH_VALS, BATCH)[None, :], (128, N_THRH * BATCH)
    ).copy()
    consts["thr_h1_rep"] = (consts["thr_h_rep"] - 1).astype(np.int16)
    consts["ones_c"] = np.ones((128, 1), np.float32)
    consts["ones_row"] = np.ones((1, 128), np.float32)
    consts["zeros_b"] = np.zeros((128, 128), ml_dtypes.bfloat16)
    consts["ident"] = np.eye(128, dtype=np.float32)
    return consts


def build_program(n_img=B_PER_CORE):
    nc = bacc.Bacc("TRN2", target_bir_lowering=False, debug=False)

    depth_in = nc.dram_tensor("depth", [n_img, H, W], F32, kind="ExternalInput").ap()
    a_in = nc.dram_tensor("a_tile", [128, W], F32, kind="ExternalInput").ap()
    bcol_in = nc.dram_tensor("b_col", [128, N_BLK], F32, kind="ExternalInput").ap()
    thrlr_in = nc.dram_tensor("thr_l_rep", [128, N_THRL * BATCH], I16, kind="ExternalInput").ap()
    thrhr_in = nc.dram_tensor("thr_h_rep", [128, N_THRH * BATCH], I16, kind="ExternalInput").ap()
    thrh1r_in = nc.dram_tensor("thr_h1_rep", [128, N_THRH * BATCH], I16, kind="ExternalInput").ap()
    ones_in = nc.dram_tensor("ones_c", [128, 1], F32, kind="ExternalInput").ap()
    onesr_in = nc.dram_tensor("ones_row", [1, 128], F32, kind="ExternalInput").ap()
    zeros_in = nc.dram_tensor("zeros_b", [128, 128], BF16, kind="ExternalInput").ap()
    ident_in = nc.dram_tensor("ident", [128, 128], F32, kind="ExternalInput").ap()
    out_dram = nc.dram_tensor("bev_out", [n_img, GRID, GRID], F32, kind="ExternalOutput").ap()

    n_repeat = int(os.environ.get("DK_REPEAT", 1))
    imgs = [i for _ in range(n_repeat) for i in range(n_img)]
    stages = [(img, V) for img in imgs for V in range(N_BLK)]
    # batch positions (within each side's issue order) offloaded to GpSimd
    pool_idx = {
        int(s) for s in os.environ.get("DK_POOL_IDX", "2,7").split(",") if s
    }

    with tile.TileContext(nc) as tc:
        with (
            tc.tile_pool(name="const", bufs=1) as cp,
            tc.tile_pool(name="work", bufs=2) as wp,
            tc.tile_pool(name="sbatch", bufs=4) as sbp,
            tc.tile_pool(name="poolsi", bufs=4) as psip,
            tc.tile_pool(name="sjpool", bufs=1) as sjp,
            tc.tile_pool(name="acc", bufs=2) as accp,
            tc.tile_pool(name="post", bufs=2) as postp,
            tc.tile_pool(name="psL", bufs=2, space="PSUM") as ppL,
            tc.tile_pool(name="psR", bufs=2, space="PSUM") as ppR,
            tc.tile_pool(name="psT", bufs=2, space="PSUM") as ppT,
            tc.tile_pool(name="psS", bufs=1, space="PSUM") as ppS,
        ):
            # ---------------- constants + stage-0 prologue head ----------
            a_t = cp.tile([128, W], F32, tag="a")
            bcol_t = cp.tile([128, N_BLK], F32, tag="b")
            ones_t = cp.tile([128, 1], F32, tag="on")
            onesr_t = cp.tile([1, 128], F32, tag="onr")
            zeros_t = cp.tile([128, 128], BF16, tag="zb")
            ident_t = cp.tile([128, 128], F32, tag="id")
            dummy_t = cp.tile([1, 1], F32, tag="dmy")
            thrl_r = cp.tile([128, N_THRL * BATCH], I16, tag="tlr")
            thrh_r = cp.tile([128, N_THRH * BATCH], I16, tag="thr")
            thrh1_r = cp.tile([128, N_THRH * BATCH], I16, tag="thr1")

            # stage-0 depth DMA first so it overlaps the const loads
            pro = {}  # stage -> dict(d, ti, tj, pool_views)
            d0 = wp.tile([128, W], F32, tag="d")
            img0, V0 = stages[0]
            nc.sync.dma_start(
                out=d0[: BLK_ROWS[V0], :],
                in_=depth_in[img0, BLK_V0[V0] : BLK_V0[V0] + BLK_ROWS[V0], :],
            )

            nc.sync.dma_start(out=bcol_t[:], in_=bcol_in[:])
            nc.sync.dma_start(out=a_t[:], in_=a_in[:])
            nc.sync.dma_start(out=thrl_r[:], in_=thrlr_in[:])
            nc.sync.dma_start(out=thrh_r[:], in_=thrhr_in[:])
            nc.sync.dma_start(out=thrh1_r[:], in_=thrh1r_in[:])
            nc.sync.dma_start(out=ones_t[:], in_=ones_in[:])
            nc.sync.dma_start(out=onesr_t[:], in_=onesr_in[:])
            nc.sync.dma_start(out=zeros_t[:], in_=zeros_in[:])
            nc.sync.dma_start(out=ident_t[:], in_=ident_in[:])
            # preload the Sqrt activation table off the critical path
            nc.vector.memset(dummy_t[:], 1.0)
            nc.scalar.activation(dummy_t[:], dummy_t[:], mybir.ActivationFunctionType.Sqrt)

            def pool_si_build(t_b, thr_v, K, wb, ge):
                """GpSimd step-matrix build: subtract + clamp in 16-u quanta
                (finer quanta shorten PE stalls).  Returns the si view."""
                tmp_b = psip.tile([128, BATCH * (N_THRL + 2)], BF16, tag="gptmp")
                tmp_v = tmp_b[:K, : BATCH * wb].rearrange("p (w c) -> p w c", c=BATCH)
                for c0 in range(0, BATCH, 16):
                    if ge:  # 1[t >= thr] = clamp(t - (thr-1), 0, 1)
                        nc.gpsimd.tensor_tensor(
                            out=tmp_v[:, :, c0 : c0 + 16],
                            in0=t_b[:, :, c0 : c0 + 16],
                            in1=thr_v[:, :, c0 : c0 + 16],
                            op=mybir.AluOpType.subtract,
                        )
                    else:  # 1[t < thr] = clamp(thr - t, 0, 1)
                        nc.gpsimd.tensor_tensor(
                            out=tmp_v[:, :, c0 : c0 + 16],
                            in0=thr_v[:, :, c0 : c0 + 16],
                            in1=t_b[:, :, c0 : c0 + 16],
                            op=mybir.AluOpType.subtract,
                        )
                    nc.gpsimd.tensor_scalar(
                        out=tmp_v[:, :, c0 : c0 + 16], in0=tmp_v[:, :, c0 : c0 + 16],
                        scalar1=0.0, scalar2=1.0,
                        op0=mybir.AluOpType.max, op1=mybir.AluOpType.min,
                    )
                return tmp_v

            def left_batches(V):
                out = []
                for bi, b0 in enumerate(range(0, U_SPLIT, BATCH)):
                    lo_min = int(I_LO[b0 : b0 + BATCH].min())
                    out.append((bi, b0, LO_BIN1 - lo_min + 1, lo_min))
                return out

            def right_batches(V):
                out = []
                for bi, b0 in enumerate(range(W - BATCH, U_SPLIT - 1, -BATCH)):
                    hi_max = int(I_HI[b0 : b0 + BATCH].max())
                    out.append((bi, b0, hi_max + 1 - HI_BIN0 + 1, hi_max))
                return out

            def emit_prologue_head(s):
                """Depth DMA + ACT tj for stage s (cheap, issue early)."""
                img, V = stages[s]
                K = BLK_ROWS[V]
                if s == 0:
                    d_t = d0
                else:
                    d_t = wp.tile([128, W], F32, tag="d")
                    nc.sync.dma_start(
                        out=d_t[:K, :], in_=depth_in[img, BLK_V0[V] : BLK_V0[V] + K, :]
                    )
                tj_t = wp.tile([128, W], I16, tag="tj")
                # ACT Copy: out = in*scale + 255.5, int16 RNE convert on write
                nc.scalar.activation(
                    tj_t[:K, :], d_t[:K, :],
                    mybir.ActivationFunctionType.Copy, bias=255.5,
                    scale=bcol_t[:K, V : V + 1],
                )
                pro[s] = {"d": d_t, "tj": tj_t}

            def emit_prologue_tail(s):
                """tif mult (DVE), ti ACT, GpSimd si prebuilds for stage s."""
                img, V = stages[s]
                K = BLK_ROWS[V]
                d_t = pro[s]["d"]
                tif_t = wp.tile([128, W], F32, tag="tif")
                nc.vector.tensor_tensor(
                    out=tif_t[:K, :], in0=d_t[:K, :], in1=a_t[:K, :],
                    op=mybir.AluOpType.mult,
                )
                ti_t = wp.tile([128, W], I16, tag="ti")
                nc.scalar.activation(
                    ti_t[:K, :], tif_t[:K, :],
                    mybir.ActivationFunctionType.Copy, bias=255.5,
                )
                pro[s]["ti"] = ti_t
                pool_views = {}
                stage_pool_idx = set() if V == 0 else pool_idx
                for bi, b0, wb, lo_min in left_batches(V):
                    if bi not in stage_pool_idx:
                        continue
                    t_b = ti_t[:K, b0 : b0 + BATCH].unsqueeze(1).to_broadcast([K, wb, BATCH])
                    thr_v = thrl_r[
                        :K, (lo_min - LO_BIN0) * BATCH : (lo_min - LO_BIN0 + wb) * BATCH
                    ].rearrange("p (w c) -> p w c", c=BATCH)
                    pool_views[("L", bi)] = pool_si_build(t_b, thr_v, K, wb, ge=False)
                for bi, b0, wb, hi_max in right_batches(V):
                    if bi not in stage_pool_idx:
                        continue
                    t_b = ti_t[:K, b0 : b0 + BATCH].unsqueeze(1).to_broadcast([K, wb, BATCH])
                    thr1_v = thrh1_r[:K, : wb * BATCH].rearrange("p (w c) -> p w c", c=BATCH)
                    pool_views[("R", bi)] = pool_si_build(t_b, thr1_v, K, wb, ge=True)
                pro[s]["pool_views"] = pool_views

            emit_prologue_head(0)
            emit_prologue_tail(0)

            NROW_L = HI_BIN0 - LO_BIN0  # 80 rows: bins [47..126]
            cur_bev = {}

            def emit_diffs(V, lt, ht, wjt, joff, on_dve=False):
                """2D finite differences for one block: i-diff along free,
                PE transpose, then j-diff along free -- no cross-partition
                shift DMA.  The last 3 left i-bins [127..129] overlap BEVR
                and are added there.  Mid-image blocks run on GpSimd; the
                image's last block runs on DVE so the stats chain does not
                queue behind GpSimd prebuilds."""
                bevl, bevr = cur_bev["l"], cur_bev["r"]
                nbj = wjt - 1  # j-bins in this block
                eng = nc.vector if on_dve else nc.gpsimd

                u_d = postp.tile([WJT_MAX, N_THRL], F32, tag="ud")
                eng.tensor_tensor(
                    out=u_d[:wjt, : N_THRL - 1],
                    in0=lt[:wjt, 1:N_THRL], in1=lt[:wjt, : N_THRL - 1],
                    op=mybir.AluOpType.subtract,
                )
                pst = ppT.tile([NBL, 2 * WJT_MAX], F32, tag="pt")
                nc.tensor.transpose(
                    out=pst[: N_THRL - 1, :wjt],
                    in_=u_d[:wjt, : N_THRL - 1],
                    identity=ident_t[:wjt, :wjt],
                )
                udT = postp.tile([NBL, WJT_MAX], F32, tag="udT")
                nc.scalar.copy(out=udT[:NROW_L, :wjt], in_=pst[:NROW_L, :wjt])
                dj = postp.tile([NBL, WJT_MAX], F32, tag="dj")
                eng.tensor_tensor(
                    out=dj[:NROW_L, :nbj],
                    in0=udT[:NROW_L, :nbj], in1=udT[:NROW_L, 1:wjt],
                    op=mybir.AluOpType.subtract,
                )
                eng.tensor_tensor(
                    out=bevl[:, joff : joff + nbj],
                    in0=bevl[:, joff : joff + nbj],
                    in1=dj[:NROW_L, :nbj],
                    op=mybir.AluOpType.add,
                )
                # overlap bins [127..129]: transpose those 3 columns to a
                # partition-0-based tile (compute slices must start 32-aligned)
                nc.tensor.transpose(
                    out=pst[0:3, WJT_MAX : WJT_MAX + wjt],
                    in_=u_d[:wjt, NROW_L : N_THRL - 1],
                    identity=ident_t[:wjt, :wjt],
                )
                udT2 = postp.tile([32, WJT_MAX], F32, tag="udT2")
                nc.scalar.copy(out=udT2[0:3, :wjt], in_=pst[0:3, WJT_MAX : WJT_MAX + wjt])
                dj2 = postp.tile([32, WJT_MAX], F32, tag="dj2")
                eng.tensor_tensor(
                    out=dj2[0:3, :nbj],
                    in0=udT2[0:3, :nbj], in1=udT2[0:3, 1:wjt],
                    op=mybir.AluOpType.subtract,
                )
                eng.tensor_tensor(
                    out=bevr[0:3, joff : joff + nbj],
                    in0=bevr[0:3, joff : joff + nbj],
                    in1=dj2[0:3, :nbj],
                    op=mybir.AluOpType.add,
                )

                u_r = postp.tile([WJT_MAX, N_THRH], F32, tag="ur")
                eng.tensor_tensor(
                    out=u_r[:wjt, : N_THRH - 1],
                    in0=ht[:wjt, : N_THRH - 1], in1=ht[:wjt, 1:N_THRH],
                    op=mybir.AluOpType.subtract,
                )
                pst_r = ppT.tile([NBR, 2 * WJT_MAX], F32, tag="pt")
                nc.tensor.transpose(
                    out=pst_r[: N_THRH - 1, :wjt],
                    in_=u_r[:wjt, : N_THRH - 1],
                    identity=ident_t[:wjt, :wjt],
                )
                urT = postp.tile([NBR, WJT_MAX], F32, tag="urT")
                nc.scalar.copy(out=urT[: N_THRH - 1, :wjt], in_=pst_r[: N_THRH - 1, :wjt])
                dj_r = postp.tile([NBR, WJT_MAX], F32, tag="djr")
                eng.tensor_tensor(
                    out=dj_r[: N_THRH - 1, :nbj],
                    in0=urT[: N_THRH - 1, :nbj], in1=urT[: N_THRH - 1, 1:wjt],
                    op=mybir.AluOpType.subtract,
                )
                eng.tensor_tensor(
                    out=bevr[:, joff : joff + nbj],
                    in0=bevr[:, joff : joff + nbj],
                    in1=dj_r[: N_THRH - 1, :nbj],
                    op=mybir.AluOpType.add,
                )

            def emit_post(img):
                """Statistics, normalize, output DMAs."""
                bevl, bevr = cur_bev["l"], cur_bev["r"]
                nrow_l = NROW_L             # 80 rows: bins [47..126]
                nrow_r = NBR                # 82 rows: bins [127..208]
                sq = postp.tile([128, NJ], F32, tag="sq")
                sql = postp.tile([128, 1], F32, tag="sql")
                sqr = postp.tile([128, 1], F32, tag="sqr")
                nc.vector.tensor_tensor(
                    out=sq[:nrow_l, :], in0=bevl[:nrow_l, :], in1=bevl[:nrow_l, :],
                    op=mybir.AluOpType.mult,
                )
                nc.vector.tensor_reduce(
                    out=sql[:nrow_l, :], in_=sq[:nrow_l, :],
                    axis=mybir.AxisListType.X, op=mybir.AluOpType.add,
                )
                nc.vector.tensor_tensor(
                    out=sq[:nrow_r, :], in0=bevr[:nrow_r, :], in1=bevr[:nrow_r, :],
                    op=mybir.AluOpType.mult,
                )
                nc.vector.tensor_reduce(
                    out=sqr[:nrow_r, :], in_=sq[:nrow_r, :],
                    axis=mybir.AxisListType.X, op=mybir.AluOpType.add,
                )
                pss = ppS.tile([1, 1], F32, tag="ps")
                nc.tensor.matmul(
                    pss[:, :], lhsT=sql[:nrow_l, :], rhs=ones_t[:nrow_l, :],
                    start=True, stop=False,
                )
                nc.tensor.matmul(
                    pss[:, :], lhsT=sqr[:nrow_r, :], rhs=ones_t[:nrow_r, :],
                    start=False, stop=True,
                )
                ib_pair = postp.tile([1, 2], F32, tag="ibp")
                var_t = postp.tile([1, 1], F32, tag="var")
                nc.vector.tensor_scalar(
                    out=var_t[:], in0=pss[:, :],
                    scalar1=-float(NVOX) * MU * MU, scalar2=1.0 / float(NVOX - 1),
                    op0=mybir.AluOpType.add, op1=mybir.AluOpType.mult,
                )
                std_t = postp.tile([1, 1], F32, tag="std")
                nc.scalar.activation(std_t[:], var_t[:], mybir.ActivationFunctionType.Sqrt)
                nc.vector.reciprocal(ib_pair[:, 0:1], std_t[:])
                nc.vector.tensor_scalar(
                    out=ib_pair[:, 1:2], in0=ib_pair[:, 0:1], scalar1=-MU, scalar2=None,
                    op0=mybir.AluOpType.mult,
                )
                # broadcast (inv, bias) across partitions via a K=1 matmul
                bc_ps = ppS.tile([128, 2], F32, tag="bc")
                nc.tensor.matmul(
                    bc_ps[:, :], lhsT=onesr_t[0:1, :], rhs=ib_pair[0:1, :],
                    start=True, stop=True,
                )
                invb = postp.tile([128, 2], F32, tag="invb")
                nc.scalar.copy(out=invb[:], in_=bc_ps[:, :])

                # ---- normalize + write out ----
                bevln = postp.tile([NBL, NJ], F32, tag="bevln")
                bevrn = postp.tile([NBR, NJ], F32, tag="bevrn")
                nc.vector.tensor_scalar(
                    out=bevln[:nrow_l, :], in0=bevl[:nrow_l, :],
                    scalar1=invb[:nrow_l, 0:1], scalar2=invb[:nrow_l, 1:2],
                    op0=mybir.AluOpType.mult, op1=mybir.AluOpType.add,
                )
                nc.vector.tensor_scalar(
                    out=bevrn[:nrow_r, :], in0=bevr[:nrow_r, :],
                    scalar1=invb[:nrow_r, 0:1], scalar2=invb[:nrow_r, 1:2],
                    op0=mybir.AluOpType.mult, op1=mybir.AluOpType.add,
                )
                border = postp.tile([128, GRID], F32, tag="border")
                nc.gpsimd.memset(border[:], 0.0)
                nc.vector.tensor_scalar(
                    out=border[:], in0=border[:], scalar1=invb[:, 1:2], scalar2=None,
                    op0=mybir.AluOpType.add,
                )

                r0 = LO_BIN0  # first computed row (47)
                r1 = HI_BIN0  # 127
                r2 = HI_BIN0 + NBR  # 209
                nc.sync.dma_start(
                    out=out_dram[img, r0:r1, JLO_G : JLO_G + NJ], in_=bevln[: r1 - r0, :]
                )
                nc.sync.dma_start(
                    out=out_dram[img, r1:r2, JLO_G : JLO_G + NJ], in_=bevrn[: r2 - r1, :]
                )

                def border_fill(rr0, rr1, cc0, cc1):
                    r = rr0
                    while r < rr1:
                        n = min(128, rr1 - r)
                        nc.sync.dma_start(
                            out=out_dram[img, r : r + n, cc0:cc1],
                            in_=border[:n, : cc1 - cc0],
                        )
                        r += n

                border_fill(0, r0, 0, GRID)
                border_fill(r2, GRID, 0, GRID)
                border_fill(r0, r2, 0, JLO_G)
                border_fill(r0, r2, JLO_G + NJ, GRID)

            # ------------------------------ main pipelined stage loop ----
            for s, (img, V) in enumerate(stages):
                if V == 0:
                    bevl = postp.tile([NROW_L, NJ], F32, tag="bevl")
                    bevr = postp.tile([NBR, NJ], F32, tag="bevr")
                    nc.gpsimd.memset(bevl[:], 0.0)
                    nc.gpsimd.memset(bevr[:], 0.0)
                    cur_bev["l"], cur_bev["r"] = bevl, bevr
                K = BLK_ROWS[V]
                wjt = J_HI[V] - J_LO[V] + 2  # j-threshold count
                joff = J_LO[V] - JLO_G       # global j offset of this block
                ti_t = pro[s]["ti"]
                tj_t = pro[s]["tj"]
                pool_views = pro[s]["pool_views"]

                # next stage's DMA + tj go out before this block's work
                if s + 1 < len(stages):
                    emit_prologue_head(s + 1)

                psL = ppL.tile([WJT_MAX, N_THRL], F32, tag="pl")
                psH = ppR.tile([WJT_MAX, N_THRH], F32, tag="pr")

                # ---- S_j build (DVE 4x), full block width per threshold ----
                sj_t = sjp.tile([128, WJT_MAX * W], BF16, tag="sj")
                sj_f = sj_t[:].rearrange("p (t u) -> p t u", u=W)
                for jt in range(wjt):
                    nc.vector.tensor_scalar(
                        out=sj_f[:K, jt, :],
                        in0=tj_t[:K, :],
                        scalar1=int(THRJ_VALS[joff + jt]),
                        scalar2=None,
                        op0=mybir.AluOpType.is_ge,
                    )

                # rest of next stage's prologue (tif/ti/pool prebuilds)
                if s + 1 < len(stages):
                    emit_prologue_tail(s + 1)

                # ---- left columns (u < U_SPLIT): S_i = (t < thr) ----
                for bi, b0, wb, lo_min in left_batches(V):
                    us = range(b0, b0 + BATCH)
                    if ("L", bi) in pool_views:
                        si_v = pool_views[("L", bi)]
                    else:
                        t_b = ti_t[:K, b0 : b0 + BATCH].unsqueeze(1).to_broadcast([K, wb, BATCH])
                        thr_v = thrl_r[
                            :K, (lo_min - LO_BIN0) * BATCH : (lo_min - LO_BIN0 + wb) * BATCH
                        ].rearrange("p (w c) -> p w c", c=BATCH)
                        si_b = sbp.tile([128, BATCH * (N_THRL + 2)], BF16, tag="si")
                        si_v = si_b[:K, : BATCH * wb].rearrange("p (w c) -> p w c", c=BATCH)
                        nc.vector.tensor_tensor(
                            out=si_v, in0=t_b, in1=thr_v, op=mybir.AluOpType.is_lt,
                        )
                    for c, u in enumerate(us):
                        wi = LO_BIN1 - int(I_LO[u]) + 1
                        foff = int(I_LO[u]) - LO_BIN0
                        nc.tensor.matmul(
                            psL[:wjt, foff : foff + wi],
                            lhsT=sj_f[:K, :wjt, u],
                            rhs=si_v[:, wb - wi :, c],
                            start=(u == 0),
                            stop=False,
                        )

                # close the left accumulation group with a full-size
                # zero matmul (stop is sim-only bookkeeping)
                nc.tensor.matmul(
                    psL[:wjt, :],
                    lhsT=zeros_t[:K, :wjt],
                    rhs=zeros_t[:K, :N_THRL],
                    start=False, stop=True,
                )

                # ---- right columns (u >= U_SPLIT), descending so the
                # widest window (u = W-1) opens the group ----
                for bi, b0, wb, hi_max in right_batches(V):
                    us = range(b0, b0 + BATCH)
                    if ("R", bi) in pool_views:
                        si_v = pool_views[("R", bi)]
                    else:
                        t_b = ti_t[:K, b0 : b0 + BATCH].unsqueeze(1).to_broadcast([K, wb, BATCH])
                        si_b = sbp.tile([128, BATCH * (N_THRL + 2)], BF16, tag="si")
                        si_v = si_b[:K, : BATCH * wb].rearrange("p (w c) -> p w c", c=BATCH)
                        nc.vector.tensor_tensor(
                            out=si_v,
                            in0=t_b,
                            in1=thrh_r[:K, : wb * BATCH].rearrange("p (w c) -> p w c", c=BATCH),
                            op=mybir.AluOpType.is_ge,
                        )
                    for cc, u in enumerate(reversed(us)):
                        c = BATCH - 1 - cc
                        wi = int(I_HI[u]) + 2 - HI_BIN0
                        nc.tensor.matmul(
                            psH[:wjt, :wi],
                            lhsT=sj_f[:K, :wjt, u],
                            rhs=si_v[:, :wi, c],
                            start=(u == W - 1),
                            stop=False,
                        )
                nc.tensor.matmul(
                    psH[:wjt, :],
                    lhsT=zeros_t[:K, :wjt],
                    rhs=zeros_t[:K, :N_THRH],
                    start=False, stop=True,
                )

                lt = accp.tile([WJT_MAX, N_THRL], F32, tag="lt")
                ht = accp.tile([WJT_MAX, N_THRH], F32, tag="ht")
                nc.scalar.copy(out=lt[:wjt, :], in_=psL[:wjt, :])
                nc.scalar.copy(out=ht[:wjt, :], in_=psH[:wjt, :])
                emit_diffs(V, lt, ht, wjt, joff, on_dve=True)

                if V == N_BLK - 1:
                    emit_post(img)

    nc.compile()
    return nc


_NC_CACHE = {}
LAST_RESULTS = None


def kernel(depth: np.ndarray) -> np.ndarray:
    """Full-input entry point: depth (16, 480, 640) f32 -> (16, 1, 256, 256) f32."""
    global LAST_RESULTS
    depth = np.asarray(depth, dtype=np.float32)
    assert depth.shape == (B_TOTAL, H, W)

    import sys, time as _time
    if "nc" not in _NC_CACHE:
        _t0 = _time.time()
        print("[kernel] building program...", file=sys.stderr, flush=True)
        _NC_CACHE["nc"] = build_program(B_PER_CORE)
        print(f"[kernel] program built in {_time.time()-_t0:.1f}s", file=sys.stderr, flush=True)
    nc = _NC_CACHE["nc"]

    consts = _make_consts()
    in_maps = []
    for c in range(N_CORES):
        m = dict(consts)
        m["depth"] = np.ascontiguousarray(depth[c * B_PER_CORE : (c + 1) * B_PER_CORE])
        in_maps.append(m)

    print("[kernel] launching spmd run...", file=__import__("sys").stderr, flush=True)
    res = run_bass_kernel_spmd(
        nc, in_maps, list(range(N_CORES)),
        trace=bool(os.environ.get("BASS_TRACE")),
    )
    LAST_RESULTS = res
    out = np.empty((B_TOTAL, 1, GRID, GRID), np.float32)
    for c in range(N_CORES):
        out[c * B_PER_CORE : (c + 1) * B_PER_CORE, 0] = res.results[c]["bev_out"]
    return out
